# revision 1
# baseline (speedup 1.0000x reference)
"""Trainium2 Bass kernel for a GPT-2 transformer block (nn_Block_29343216566701).

Sharding: data-parallel over batch B=8 -> 8 NeuronCores, one batch element per
core, no collectives. Each core runs the full block on [1024 tokens, 768 feats].

On-chip layout is feature-major (x^T: [768, 1024] as [128, 6, 1024] SBUF tiles)
so every matmul contracts over the partition dim without transposes:
  - qkv:   out = W^T @ x^T  (lhsT = W as stored, rhs = x^T)       [feature-major]
  - V:     out = x^T^T @ Wv (lhsT = x^T tile, rhs = Wv)           [token-major]
  - scores:s^T = K^T^T?? -> lhsT = k^T head slice, rhs = q^T head [k x q]
  - attn:  y' = V'(token-major, +ones col)^T.T @ exp(s^T)         [d+1 x q]
  - denominators come from the ones column; 1/x via ACT exp(-ln(x)).
Precision: bf16 matmul inputs with f32 PSUM accumulation; f32 residual spine;
fp32r (1 cycle/row, ~12-bit mantissa) for LN stat matmuls; LN gamma/beta folded
into the following weight matrix host-side (exact for this block).
"""
import numpy as np
import ml_dtypes

import concourse.bass as bass
import concourse.tile as tile
import concourse.mybir as mybir
from concourse.bass_utils import run_bass_kernel_spmd
from concourse.vector_clock import ScopedClock

F32 = mybir.dt.float32
F32R = mybir.dt.float32r
BF16 = mybir.dt.bfloat16
AF = mybir.ActivationFunctionType
OP = mybir.AluOpType
BF = ml_dtypes.bfloat16

B, T, C = 8, 1024, 768
H, HD = 12, 64
NC = C // 128          # 6 feature chunks
NT = T // 128          # 8 token tiles
FF = 4 * C             # 3072
NF = FF // 128         # 24
LN_EPS = 1e-5


# ---------------------------------------------------------------------------
# walrus codegen accepts only one fused semaphore wait per instruction; hoist
# excess waits onto preceding nofuse NOPs on the same engine.
def _split_excess_waits(nc, cap=1):
    for fn in nc.m.functions:
        for bb in fn.blocks:
            new = []
            changed = False
            for ins in bb.instructions:
                si = getattr(ins, "sync_info", None)
                waits = list(si.on_wait) if (si is not None and si.on_wait) else []
                if len(waits) > cap:
                    changed = True
                    for i, w in enumerate(waits[:-cap]):
                        new.append(mybir.InstNoOp(
                            name=f"{ins.name}-w{i}",
                            engine=ins.engine,
                            sync_info=mybir.SyncInfo(on_wait=[w], on_update=[]),
                            bass_nofuse=True,
                        ))
                    ins.sync_info = mybir.SyncInfo(
                        on_wait=waits[-cap:], on_update=list(si.on_update))
                new.append(ins)
            if changed:
                bb.instructions = new


class _SplitDrainTC(tile.TileContext):
    """TileContext whose kernel-tail drain carries its waits on single-wait
    NOPs (the stock version fuses them all onto one drain instruction)."""

    def _drain_and_barrier(self, tick_clock, wait_clock):
        nc = self.nc
        probe = nc.sync.nop(nofuse=True, hint="tail_wait0")
        wait_clock.add_sem_waits(
            probe.ins, ScopedClock({None: tick_clock.global_clock}))
        waits = list(probe.ins.sync_info.on_wait) if probe.ins.sync_info else []
        if len(waits) > 1:
            probe.ins.sync_info = mybir.SyncInfo(on_wait=waits[:1], on_update=[])
            for i, w in enumerate(waits[1:]):
                n = nc.sync.nop(nofuse=True, hint=f"tail_wait{i + 1}")
                n.ins.sync_info = mybir.SyncInfo(on_wait=[w], on_update=[])
        nc.sync.drain()
        nc.all_engine_barrier()
        assert self.sems is not None
        popped = nc._tile_sem_poison_stack.pop()
        assert popped is self._sem_poison
        nc.clear_and_free_semaphores(list(self.sems.allocated().values()))
        nc.all_engine_barrier()


# ---------------------------------------------------------------------------
PHASE_MARKS = []


def _mark(nc, label):
    if not any(l == label for l, _ in PHASE_MARKS):
        PHASE_MARKS.append((label, len(nc.inst_map)))


def _build(nrep=1, loop_n=0, skip_mlp=False, skip_attn=False, probe_nowdma=False, probe_nodve=False):
    nc = bass.Bass(trn_type="TRN2", name="gpt2block")

    xT = nc.dram_tensor("xT", [C, T], F32, kind="ExternalInput")
    wqk = nc.dram_tensor("wqk", [2 * NC, 128, C], BF16, kind="ExternalInput")
    wv = nc.dram_tensor("wv", [C, C], BF16, kind="ExternalInput")
    bqk = nc.dram_tensor("bqk", [2 * C], F32, kind="ExternalInput")
    bv = nc.dram_tensor("bv", [1, C], BF16, kind="ExternalInput")
    wproj = nc.dram_tensor("wproj", [NC, 128, C], BF16, kind="ExternalInput")
    bproj = nc.dram_tensor("bproj", [C], F32, kind="ExternalInput")
    wfc = nc.dram_tensor("wfc", [NF, 128, C], BF16, kind="ExternalInput")
    bfc = nc.dram_tensor("bfc", [FF], F32, kind="ExternalInput")
    wfc2 = nc.dram_tensor("wfc2", [NC, 128, FF], BF16, kind="ExternalInput")
    bfc2 = nc.dram_tensor("bfc2", [C], F32, kind="ExternalInput")
    outT = nc.dram_tensor("outT", [C, T], F32, kind="ExternalOutput")

    # causal handling for diagonal 128x128 blocks of s^T[k, q]: accumulate
    # -1e9 where q < k via an identity matmul, so exp underflows to exact 0
    ident_d = nc.inline_tensor(np.eye(128).astype(BF), name="ident")
    mneg_d = nc.inline_tensor(
        (-1e9 * np.tril(np.ones((128, 128)), -1)).astype(BF), name="mneg")

    with _SplitDrainTC(nc) as tc:
        with tc.tile_pool(name="persist", bufs=1) as pp, \
             tc.tile_pool(name="big", bufs=2) as bigp, \
             tc.tile_pool(name="t32p", bufs=2) as t32p, \
             tc.tile_pool(name="frp", bufs=2) as frp, \
             tc.tile_pool(name="tep", bufs=3) as tep, \
             tc.tile_pool(name="ptp", bufs=4) as ptp, \
             tc.tile_pool(name="wp", bufs=4) as wp, \
             tc.tile_pool(name="ps", bufs=2, space="PSUM") as ps:

            # ---------------- constants / small inputs ----------------
            ident_sb = pp.tile([128, 128], BF16, tag="ident")
            nc.sync.dma_start(ident_sb[:], ident_d[:])
            mneg_sb = pp.tile([128, 128], BF16, tag="mneg")
            nc.sync.dma_start(mneg_sb[:], mneg_d[:])
            ones512 = pp.tile([128, 512], BF16, tag="ones512")
            nc.vector.memset(ones512[:], 1.0)
            j32 = t32p.tile([128, 128], F32, tag="t32", name="j32")
            nc.vector.memset(j32[:], 1.0 / C)
            jr = pp.tile([128, 128], F32R, tag="jr")
            nc.vector.tensor_copy(jr[:], j32[:])
            o32 = t32p.tile([128, 128], F32, tag="t32", name="o32")
            nc.vector.memset(o32[:], 1.0)
            o64r = pp.tile([128, 128], F32R, tag="o64r")
            nc.vector.tensor_copy(o64r[:], o32[:])
            eps_sb = pp.tile([128, 1], F32, tag="eps")
            nc.vector.memset(eps_sb[:], LN_EPS)

            bqksb = pp.tile([128, 2 * NC], F32, tag="bqksb")
            nc.sync.dma_start(bqksb[:], bqk.rearrange("(c p) -> p c", p=128))
            bfcsb = pp.tile([128, NF], F32, tag="bfcsb")
            nc.sync.dma_start(bfcsb[:], bfc.rearrange("(c p) -> p c", p=128))
            bvsb = pp.tile([1, C], BF16, tag="bvsb")
            nc.sync.dma_start(bvsb[:], bv[:])
            bprojsb = pp.tile([128, NC], F32, tag="bprojsb")
            nc.sync.dma_start(bprojsb[:], bproj.rearrange("(c p) -> p c", p=128))
            bfc2sb = pp.tile([128, NC], F32, tag="bfc2sb")
            nc.sync.dma_start(bfc2sb[:], bfc2.rearrange("(c p) -> p c", p=128))

            import contextlib
            loop_cm = (tc.For_i(0, loop_n, 1) if loop_n
                       else contextlib.nullcontext())
            with loop_cm:
              for _rep in range(nrep):
                # ---------------- big persistent activations ----------------
                xts = bigp.tile([128, NC, T], F32, tag="big")      # x^T
                xTv = xT.rearrange("(c p) t -> p c t", p=128)
                for c in range(NC):
                    nc.sync.dma_start(xts[:, c, :], xTv[:, c, :])
                x1 = pp.tile([128, NC, T], F32, tag="x1")          # residual after attn
                lnout = pp.tile([128, NC, T], BF16, tag="lnout")   # LN output (reused)
                qT = pp.tile([128, NC, T], BF16, tag="qT")
                kT = pp.tile([128, NC, T], BF16, tag="kT")
                yT = pp.tile([128, NC, T], BF16, tag="yT")
                vsb = pp.tile([128, NT, H, HD + 1], BF16, tag="vsb")
                nc.vector.memset(vsb[:, :, :, HD:HD + 1], 1.0)
                wvsb = pp.tile([128, NC, C], BF16, tag="wvsb")
                nc.sync.dma_start(wvsb[:], wv.rearrange("(c p) v -> p c v", p=128))
                mu_sb = pp.tile([128, T], F32, tag="mu")
                rstd_sb = pp.tile([128, T], F32, tag="rstd")

                # ---------------- helpers ----------------
                def layernorm(src, dst, cen_eng=None):
                    """dst (bf16) = (src - mean_f) * rsqrt(var_f + eps); feature
                    (=partition+chunk) reductions via fp32r matmuls with an
                    all-ones/C stationary -> partition-broadcast mean directly.
                    The var chain and normalize run per q-half to shorten the
                    critical path to the first consumable dst columns."""
                    MU = ps.tile([128, T], F32, tag="mm", name="MUp")
                    SSQ = ps.tile([128, T], F32, tag="acc", name="SSQp")
                    for c in range(NC):
                        xr = frp.tile([128, T], F32R, tag="fr", name=f"xr{c}")
                        x2r = frp.tile([128, T], F32R, tag="fr", name=f"x2r{c}")
                        for n0 in (0, 512):
                            nc.vector.tensor_copy(xr[:, n0:n0 + 512],
                                                  src[:, c, n0:n0 + 512])
                            nc.scalar.activation(x2r[:, n0:n0 + 512],
                                                 src[:, c, n0:n0 + 512], AF.Square)
                            nc.tensor.matmul(MU[:, n0:n0 + 512], jr[:],
                                             xr[:, n0:n0 + 512],
                                             start=(c == 0), stop=(c == NC - 1))
                            nc.tensor.matmul(SSQ[:, n0:n0 + 512], jr[:],
                                             x2r[:, n0:n0 + 512],
                                             start=(c == 0), stop=(c == NC - 1))
                    for n0 in (0, 512):
                        hs = slice(n0, n0 + 512)
                        nc.vector.tensor_copy(mu_sb[:, hs], MU[:, hs])
                        musq = t32p.tile([128, 512], F32, tag="th", name=f"musq{n0}")
                        nc.vector.tensor_tensor(musq[:], mu_sb[:, hs], mu_sb[:, hs],
                                                op=OP.mult)
                        var = t32p.tile([128, 512], F32, tag="th", name=f"var{n0}")
                        nc.vector.tensor_tensor(var[:], SSQ[:, hs], musq[:],
                                                op=OP.subtract)
                        lnv = t32p.tile([128, 512], F32, tag="th", name=f"lnv{n0}")
                        nc.scalar.activation(lnv[:], var[:], AF.Ln, bias=eps_sb[:])
                        nc.scalar.activation(rstd_sb[:, hs], lnv[:], AF.Exp,
                                             scale=-0.5)
                        for c in range(NC):
                            cen = t32p.tile([128, 512], F32, tag="th",
                                            name=f"cen{n0}_{c}")
                            (cen_eng or nc.gpsimd).tensor_tensor(
                                cen[:], src[:, c, hs],
                                mu_sb[:, hs], op=OP.subtract)
                            nc.vector.tensor_tensor(dst[:, c, hs], cen[:],
                                                    rstd_sb[:, hs], op=OP.mult)

                _mark(nc, 'ln1')
                layernorm(xts, lnout)

                # ---------------- phase 2+3: qkv interleaved with attention ----
                _mark(nc, 'V')
                for ti in range(NT):
                    pm = ps.tile([128, T], F32, tag="mm" if ti % 2 == 0 else "acc", name=f"vp{ti}")
                    for k in range(NC):
                        lh = lnout[:, k, ti * 128:(ti + 1) * 128]
                        nc.tensor.matmul(pm[:, 0:512], lh, wvsb[:, k, 0:512],
                                         start=(k == 0), stop=False)
                        nc.tensor.matmul(pm[:, 512:768], lh, wvsb[:, k, 512:768],
                                         start=(k == 0), stop=False)
                    nc.tensor.matmul(pm[:, 0:512], ones512[0:1, 0:128],
                                     bvsb[0:1, 0:512], start=False, stop=True)
                    nc.tensor.matmul(pm[:, 512:768], ones512[0:1, 0:128],
                                     bvsb[0:1, 512:768], start=False, stop=True)
                    nc.scalar.activation(
                        vsb[:, ti, :, 0:HD],
                        pm[:, 0:768].rearrange("p (h d) -> p h d", h=H),
                        AF.Copy)

                def qk_chunk(mi):
                    # Q^T / K^T feature chunk (feature-major): lhsT = W tile
                    pm = ps.tile([128, T], F32, tag="mm", name=f"qkp{mi}")
                    wt = wp.tile([128, NC, 128], BF16, tag="w", name=f"wqk{mi}")
                    nc.sync.dma_start(wt[:], wqk[mi].rearrange("p (c m) -> p c m", c=NC))
                    for k in range(NC):
                        for n0 in (0, 512):
                            nc.tensor.matmul(pm[:, n0:n0 + 512], wt[:, k, :],
                                             lnout[:, k, n0:n0 + 512],
                                             start=(k == 0), stop=(k == NC - 1))
                    dstt = qT if mi < NC else kT
                    nc.vector.tensor_scalar_add(
                        dstt[:, mi % NC, :], pm[:], bqksb[:, mi:mi + 1])

                def attn_pair(p, yPa, yPb):
                    # two heads (2p: partitions 0:64, 2p+1: 64:128) interleaved
                    # per k-tile: their score matmuls land in different PE row
                    # groups and run concurrently; exp of one head overlaps the
                    # other head's matmuls. For the short k-tiles (qlen<=512)
                    # both heads' scores share one PSUM tile and one exp call.
                    ch = p
                    for kt in range(NT):
                        qs = 128 * kt
                        qlen = T - qs
                        sts, pts = [], []
                        if qlen <= 512:
                            sT2 = ps.tile([128, 2, 512], F32, tag="mm",
                                          name=f"sT2{p}_{kt}")
                            pt2 = ptp.tile([128, 2, 512], BF16, tag="pt",
                                           name=f"pt2{p}_{kt}")
                            for i, p0 in enumerate((0, 64)):
                                lh = kT[p0:p0 + 64, ch, qs:qs + 128]
                                nc.tensor.matmul(sT2[:, i, 0:qlen], lh,
                                                 qT[p0:p0 + 64, ch, qs:T],
                                                 start=True, stop=False)
                                nc.tensor.matmul(sT2[:, i, 0:128], ident_sb[:],
                                                 mneg_sb[:], start=False,
                                                 stop=True)
                            nc.scalar.activation(pt2[:, :, 0:qlen],
                                                 sT2[:, :, 0:qlen], AF.Exp,
                                                 scale=float(1.0 / np.sqrt(HD)))
                            for i in range(2):
                                pts.append(pt2[:, i, :])
                        else:
                          for i, p0 in enumerate((0, 64)):
                            sT = ps.tile([128, T], F32, tag="mm",
                                         name=f"sT{p}_{i}_{kt}")
                            lh = kT[p0:p0 + 64, ch, qs:qs + 128]
                            for a in range(0, qlen, 512):
                                bnd = min(a + 512, qlen)
                                nc.tensor.matmul(sT[:, a:bnd], lh,
                                                 qT[p0:p0 + 64, ch,
                                                    qs + a:qs + bnd],
                                                 start=True, stop=(a > 0))
                            nc.tensor.matmul(sT[:, 0:128], ident_sb[:],
                                             mneg_sb[:], start=False,
                                             stop=True)
                            sts.append(sT)
                          for i in range(2):
                            pt = ptp.tile([128, T], BF16, tag="pt",
                                          name=f"pt{p}_{i}_{kt}")
                            nc.scalar.activation(pt[:, 0:qlen], sts[i][:, 0:qlen],
                                                 AF.Exp,
                                                 scale=float(1.0 / np.sqrt(HD)))
                            pts.append(pt)
                        for i, yP in enumerate((yPa, yPb)):
                            h = 2 * p + i
                            vl = vsb[:, kt, h, :]
                            pt = pts[i]
                            if qs < 512:
                                nc.tensor.matmul(yP[0:HD + 1, qs:512], vl,
                                                 pt[:, 0:512 - qs],
                                                 start=(kt == 0), stop=(kt == 3))
                                nc.tensor.matmul(yP[0:HD + 1, 512:T], vl,
                                                 pt[:, 512 - qs:T - qs],
                                                 start=(kt == 0),
                                                 stop=(kt == NT - 1))
                            else:
                                nc.tensor.matmul(yP[0:HD + 1, qs:T], vl,
                                                 pt[:, 0:qlen],
                                                 start=False, stop=(kt == NT - 1))

                def attn_norm(h, yP):
                    # rec = exp(-ln(denominator)); broadcast over 64 partitions
                    # via a K=1 fp32r matmul; y = y' * rec
                    p0 = 64 * (h % 2)
                    ch = h // 2
                    lnden = t32p.tile([128, T], F32, tag="t32", name=f"lnden{h}")
                    nc.scalar.activation(lnden[HD:HD + 1, :], yP[HD:HD + 1, :], AF.Ln)
                    rec = frp.tile([128, T], F32R, tag="fr", name=f"rec{h}")
                    nc.scalar.activation(rec[HD:HD + 1, :], lnden[HD:HD + 1, :],
                                         AF.Exp, scale=-1.0)
                    R64 = ps.tile([128, T], F32, tag="mm", name=f"r64_{h}")
                    for n0 in (0, 512):
                        nc.tensor.matmul(R64[0:64, n0:n0 + 512],
                                         o64r[HD:HD + 1, 0:64],
                                         rec[HD:HD + 1, n0:n0 + 512],
                                         start=True, stop=True)
                    r64sb = t32p.tile([128, T], F32, tag="t32", name=f"r64sb{h}")
                    nc.vector.tensor_copy(r64sb[0:64, :], R64[0:64, :])
                    if p0 == 0:
                        nc.vector.tensor_tensor(yT[0:64, ch, :], yP[0:64, :],
                                                r64sb[0:64, :], op=OP.mult)
                    else:
                        yo = ptp.tile([128, T], BF16, tag="pt", name=f"yo{h}")
                        nc.vector.tensor_tensor(yo[0:64, :], yP[0:64, :],
                                                r64sb[0:64, :], op=OP.mult)
                        nc.sync.dma_start(yT[64:128, ch, :], yo[0:64, :])

                # interleave: QK chunk pair p, then attention heads 2p / 2p+1 —
                # the next pair's QK matmuls keep PE busy while ACT runs exp
                if skip_attn:
                    nc.vector.memset(yT[:], 0.001)
                pending = None
                for p in range(() if skip_attn else range(NC)) if False else (range(0) if skip_attn else range(NC)):
                    _mark(nc, f'pair{p}')
                    qk_chunk(p)       # Q chunk p
                    qk_chunk(NC + p)  # K chunk p
                    # normalization of the previous pair lands here so its
                    # Ln/Exp chain overlaps this pair's QK matmuls on PE
                    if pending is not None:
                        attn_norm(pending[0], pending[1])
                        attn_norm(pending[0] + 1, pending[2])
                    yPa = ps.tile([128, T], F32, tag="acc", name=f"yp{2 * p}")
                    yPb = ps.tile([128, T], F32, tag="acc", name=f"yp{2 * p + 1}")
                    attn_pair(p, yPa, yPb)
                    pending = (2 * p, yPa, yPb)
                if pending is not None:
                    attn_norm(pending[0], pending[1])
                    attn_norm(pending[0] + 1, pending[2])

                _mark(nc, 'proj')
                for m in range(NC):
                    pm = ps.tile([128, T], F32, tag="mm" if m % 2 == 0 else "acc", name=f"pj{m}")
                    wt = wp.tile([128, NC, 128], BF16, tag="w", name=f"wpj{m}")
                    nc.sync.dma_start(wt[:], wproj[m].rearrange("p (c m) -> p c m", c=NC))
                    for k in range(NC):
                        for n0 in (0, 512):
                            nc.tensor.matmul(pm[:, n0:n0 + 512], wt[:, k, :],
                                             yT[:, k, n0:n0 + 512],
                                             start=(k == 0), stop=(k == NC - 1))
                    t = tep.tile([128, T], F32, tag="te", name=f"pjt{m}")
                    nc.scalar.activation(t[:], pm[:], AF.Identity,
                                         bias=bprojsb[:, m:m + 1])
                    nc.vector.tensor_tensor(x1[:, m, :], t[:], xts[:, m, :],
                                            op=OP.add)

                _mark(nc, 'ln2')
                layernorm(x1, lnout, cen_eng=nc.vector)

                # ---------------- phase 6/7: MLP ----------------
                if skip_mlp:
                    for m in range(NC):
                        nc.sync.dma_start(outT[m * 128:(m + 1) * 128, :],
                                          x1[:, m, :])
                for half in range(0 if skip_mlp else 2):
                    _mark(nc, f'mlp{half}')
                    h2 = bigp.tile([128, NF // 2, T], BF16, tag="big",
                                   name=f"h2_{half}")
                    wt_probe = None
                    for m in range(NF // 2):
                        mg = half * (NF // 2) + m
                        pm = ps.tile([128, T], F32, tag="mm" if mg % 2 == 0 else "acc", name=f"fc{mg}")
                        if probe_nowdma and wt_probe is not None:
                            wt = wt_probe
                        else:
                            wt = wp.tile([128, NC, 128], BF16, tag="w", name=f"wfc{mg}")
                            nc.sync.dma_start(
                                wt[:], wfc[mg].rearrange("p (c m) -> p c m", c=NC))
                            wt_probe = wt
                        for k in range(NC):
                            for n0 in (0, 512):
                                nc.tensor.matmul(pm[:, n0:n0 + 512], wt[:, k, :],
                                                 lnout[:, k, n0:n0 + 512],
                                                 start=(k == 0), stop=(k == NC - 1))
                        nc.scalar.activation(h2[:, m, :], pm[:], AF.Gelu,
                                             bias=bfcsb[:, mg:mg + 1])
                    wt2_probe = None
                    for m in range(NC):
                        pm = ps.tile([128, T], F32, tag="mm" if m % 2 == 0 else "acc", name=f"fc2_{half}_{m}")
                        if probe_nowdma and wt2_probe is not None:
                            wt = wt2_probe
                        else:
                            wt = wp.tile([128, NF // 2, 128], BF16, tag="w",
                                         name=f"wfc2_{half}_{m}")
                            nc.sync.dma_start(
                                wt[:],
                                wfc2[m, :, half * 1536:(half + 1) * 1536]
                                .rearrange("p (c m) -> p c m", c=NF // 2))
                            wt2_probe = wt
                        last = NF // 2 - 1
                        for k in range(NF // 2):
                            for n0 in (0, 512):
                                nc.tensor.matmul(pm[:, n0:n0 + 512], wt[:, k, :],
                                                 h2[:, k, n0:n0 + 512],
                                                 start=(k == 0),
                                                 stop=(k == last))
                        t = tep.tile([128, T], F32, tag="te",
                                      name=f"f2t{half}_{m}")
                        if half == 0:
                            nc.scalar.activation(t[:], pm[:], AF.Identity,
                                                 bias=bfc2sb[:, m:m + 1])
                        else:
                            nc.scalar.activation(t[:], pm[:], AF.Identity)
                        nc.vector.tensor_tensor(x1[:, m, :], t[:], x1[:, m, :],
                                                op=OP.add)
                        if half == 1:
                            nc.sync.dma_start(outT[m * 128:(m + 1) * 128, :],
                                              x1[:, m, :])


    _split_excess_waits(nc)
    return nc


_STATE = {}


def kernel(**inputs):
    x = np.asarray(inputs["x"], np.float32)
    ln1_g = np.asarray(inputs["ln1_g"], np.float32)
    ln1_b = np.asarray(inputs["ln1_b"], np.float32)
    ln2_g = np.asarray(inputs["ln2_g"], np.float32)
    ln2_b = np.asarray(inputs["ln2_b"], np.float32)
    W_attn = np.asarray(inputs["W_attn"], np.float32)
    b_attn = np.asarray(inputs["b_attn"], np.float32)
    W_proj = np.asarray(inputs["W_proj"], np.float32)
    b_proj = np.asarray(inputs["b_proj"], np.float32)
    W_fc = np.asarray(inputs["W_fc"], np.float32)
    b_fc = np.asarray(inputs["b_fc"], np.float32)
    W_fc2 = np.asarray(inputs["W_fc2"], np.float32)
    b_fc2 = np.asarray(inputs["b_fc2"], np.float32)

    # fold LN affine into the following matmul (exact): (n*g + b) @ W
    Wa = W_attn * ln1_g[:, None]
    ba = b_attn + ln1_b @ W_attn
    Wf = W_fc * ln2_g[:, None]
    bf = b_fc + ln2_b @ W_fc

    def blk(w):
        # [K, M] -> [M/128 blocks][128 kp][K/128 * 128 mp] with feature
        # f = 128*kc + kp on the partition axis
        K, M = w.shape
        return np.ascontiguousarray(
            w.astype(BF).reshape(K // 128, 128, M // 128, 128)
            .transpose(2, 1, 0, 3).reshape(M // 128, 128, K))

    shared = {
        "wqk": blk(Wa[:, :2 * C]),
        "wv": np.ascontiguousarray(Wa[:, 2 * C:].astype(BF)),
        "bqk": np.ascontiguousarray(ba[:2 * C]),
        "bv": np.ascontiguousarray(ba[None, 2 * C:].astype(BF)),
        "wproj": blk(W_proj),
        "bproj": np.ascontiguousarray(b_proj),
        "wfc": blk(Wf),
        "bfc": np.ascontiguousarray(bf),
        "wfc2": blk(W_fc2),
        "bfc2": np.ascontiguousarray(b_fc2),
    }
    in_maps = []
    for b in range(B):
        m = dict(shared)
        m["xT"] = np.ascontiguousarray(x[b].T)
        in_maps.append(m)

    if "nc" not in _STATE:
        _STATE["nc"] = _build()
    global _last_in_maps
    _last_in_maps = in_maps
    res = run_bass_kernel_spmd(_STATE["nc"], in_maps, core_ids=list(range(B)))
    out = np.stack([r["outT"].T for r in res.results])
    return np.ascontiguousarray(out, dtype=np.float32)



# revision 30
# speedup vs baseline: 1.1514x; 1.1514x over previous
"""Trainium2 Bass kernel for a GPT-2 transformer block (nn_Block_29343216566701).

Sharding: data-parallel over batch B=8 -> 8 NeuronCores, one batch element per
core, no collectives. Each core runs the full block on [1024 tokens, 768 feats].

On-chip layout is feature-major (x^T: [768, 1024] as [128, 6, 1024] SBUF tiles)
so every matmul contracts over the partition dim without transposes:
  - qkv:   out = W^T @ x^T  (lhsT = W as stored, rhs = x^T)       [feature-major]
  - V:     out = x^T^T @ Wv (lhsT = x^T tile, rhs = Wv)           [token-major]
  - scores:s^T[k,q] (lhsT = k^T head slice, rhs = q^T head)       [k x q]
  - attn:  y' = V'(token-major, +ones col)^T.T @ exp(s^T)         [d+1 x q]
  - denominators come from the ones column; 1/x via DVE reciprocal.
Precision: bf16 matmul inputs with f32 PSUM accumulation; f32 residual spine;
fp32r (1 cycle/row, ~12-bit mantissa) for LN stat matmuls; LN gamma/beta folded
into the following weight matrix host-side (exact for this block).
Engine balance: LN squares + causal mask (0/1 multiply post-exp) on GPSIMD;
V bias via precomputed SBUF tile + DVE add; LN stat matmuls read activation
tiles directly (bf16 or f32 bitcast to f32r) with no staging copies.
A bf16 copy of x is DMA'd first so LN1 stats start ~2x sooner; the f32 x for
the residual spine streams in behind it.
"""
import numpy as np
import ml_dtypes

import concourse.bass as bass
import concourse.tile as tile
import concourse.mybir as mybir
from concourse.bass_utils import run_bass_kernel_spmd
from concourse.vector_clock import ScopedClock

F32 = mybir.dt.float32
F32R = mybir.dt.float32r
BF16 = mybir.dt.bfloat16
FP8 = mybir.dt.float8e4
AF = mybir.ActivationFunctionType
OP = mybir.AluOpType
BF = ml_dtypes.bfloat16
F8 = ml_dtypes.float8_e4m3
MM8 = mybir.MatmulPerfMode.DoubleRow
ACT_SCALE = 16.0          # fp8 activation pre-scale (lnout2, attention v)

B, T, C = 8, 1024, 768
H, HD = 12, 64
NC = C // 128          # 6 feature chunks
NT = T // 128          # 8 token tiles
FF = 4 * C             # 3072
NF = FF // 128         # 24
LN_EPS = 1e-5


# ---------------------------------------------------------------------------
# walrus codegen accepts only one fused semaphore wait per instruction; hoist
# excess waits onto preceding nofuse NOPs on the same engine.
def _split_excess_waits(nc, cap=1):
    for fn in nc.m.functions:
        for bb in fn.blocks:
            new = []
            changed = False
            for ins in bb.instructions:
                si = getattr(ins, "sync_info", None)
                waits = list(si.on_wait) if (si is not None and si.on_wait) else []
                if len(waits) > cap:
                    changed = True
                    for i, w in enumerate(waits[:-cap]):
                        new.append(mybir.InstNoOp(
                            name=f"{ins.name}-w{i}",
                            engine=ins.engine,
                            sync_info=mybir.SyncInfo(on_wait=[w], on_update=[]),
                            bass_nofuse=True,
                        ))
                    ins.sync_info = mybir.SyncInfo(
                        on_wait=waits[-cap:], on_update=list(si.on_update))
                new.append(ins)
            if changed:
                bb.instructions = new


class _SplitDrainTC(tile.TileContext):
    """TileContext whose kernel-tail drain carries its waits on single-wait
    NOPs (the stock version fuses them all onto one drain instruction)."""

    def _drain_and_barrier(self, tick_clock, wait_clock):
        nc = self.nc
        probe = nc.sync.nop(nofuse=True, hint="tail_wait0")
        wait_clock.add_sem_waits(
            probe.ins, ScopedClock({None: tick_clock.global_clock}))
        waits = list(probe.ins.sync_info.on_wait) if probe.ins.sync_info else []
        if len(waits) > 1:
            probe.ins.sync_info = mybir.SyncInfo(on_wait=waits[:1], on_update=[])
            for i, w in enumerate(waits[1:]):
                n = nc.sync.nop(nofuse=True, hint=f"tail_wait{i + 1}")
                n.ins.sync_info = mybir.SyncInfo(on_wait=[w], on_update=[])
        nc.sync.drain()
        nc.all_engine_barrier()
        assert self.sems is not None
        popped = nc._tile_sem_poison_stack.pop()
        assert popped is self._sem_poison
        nc.clear_and_free_semaphores(list(self.sems.allocated().values()))
        nc.all_engine_barrier()


# ---------------------------------------------------------------------------
PHASE_MARKS = []


def _mark(nc, label):
    if not any(l == label for l, _ in PHASE_MARKS):
        PHASE_MARKS.append((label, len(nc.inst_map)))


def _build(nrep=1, loop_n=0, s_fc=4096.0, s_fc2=8192.0):
    nc = bass.Bass(trn_type="TRN2", name="gpt2block")

    xT = nc.dram_tensor("xT", [C, T], F32R, kind="ExternalInput")
    wqk = nc.dram_tensor("wqk", [2 * NC, 128, C], BF16, kind="ExternalInput")
    wv = nc.dram_tensor("wv", [C, C], BF16, kind="ExternalInput")
    bqk = nc.dram_tensor("bqk", [2 * C], F32, kind="ExternalInput")
    bv = nc.dram_tensor("bv", [1, C], BF16, kind="ExternalInput")
    wproj = nc.dram_tensor("wproj", [NC, 128, C], BF16, kind="ExternalInput")
    bproj = nc.dram_tensor("bproj", [C], F32, kind="ExternalInput")
    wfc = nc.dram_tensor("wfc", [NF, 128, C], FP8, kind="ExternalInput")
    bfc = nc.dram_tensor("bfc", [FF], F32, kind="ExternalInput")
    wfc2 = nc.dram_tensor("wfc2", [NC, 128, FF], FP8, kind="ExternalInput")
    bfc2 = nc.dram_tensor("bfc2", [C], F32, kind="ExternalInput")
    outT = nc.dram_tensor("outT", [C, T], F32, kind="ExternalOutput")

    # causal keep-mask for the diagonal 128x128 block of s^T[k, q]:
    # keep q >= k (upper triangle incl. diagonal); applied post-exp on GPSIMD
    trimask_d = nc.inline_tensor(
        np.triu(np.ones((128, 128))).astype(BF), name="trimask")

    with _SplitDrainTC(nc) as tc:
        with tc.tile_pool(name="persist", bufs=1) as pp, \
             tc.tile_pool(name="big", bufs=2) as bigp, \
             tc.tile_pool(name="t32p", bufs=2) as t32p, \
             tc.tile_pool(name="frp", bufs=2) as frp, \
             tc.tile_pool(name="tep", bufs=3) as tep, \
             tc.tile_pool(name="ptp", bufs=4) as ptp, \
             tc.tile_pool(name="wp", bufs=4) as wp, \
             tc.tile_pool(name="ps", bufs=2, space="PSUM") as ps:

            # ---------------- constants / small inputs ----------------
            trimask = pp.tile([128, 128], BF16, tag="trimask")
            nc.sync.dma_start(trimask[:], trimask_d[:])
            ones1 = pp.tile([1, 128], BF16, tag="ones1")
            nc.vector.memset(ones1[:], 1.0)
            j32 = t32p.tile([128, 128], F32, tag="t32", name="j32")
            nc.vector.memset(j32[:], 1.0 / C)
            jr = pp.tile([128, 128], F32R, tag="jr")
            nc.vector.tensor_copy(jr[:], j32[:])
            o32 = t32p.tile([128, 128], F32, tag="t32", name="o32")
            nc.vector.memset(o32[:], 1.0)
            o64r = pp.tile([128, 128], F32R, tag="o64r")
            nc.vector.tensor_copy(o64r[:], o32[:])
            eps_sb = pp.tile([128, 1], F32, tag="eps")
            nc.vector.memset(eps_sb[:], LN_EPS)
            ln16_sb = pp.tile([128, 1], F32, tag="ln16")
            nc.vector.memset(ln16_sb[:], float(np.log(ACT_SCALE)))

            bqksb = pp.tile([128, 2 * NC], F32, tag="bqksb")
            nc.sync.dma_start(bqksb[:], bqk.rearrange("(c p) -> p c", p=128))
            bfcsb = pp.tile([128, NF], F32, tag="bfcsb")
            nc.sync.dma_start(bfcsb[:], bfc.rearrange("(c p) -> p c", p=128))
            bvsb = t32p.tile([1, C], BF16, tag="t32", name="bvsb")
            nc.sync.dma_start(bvsb[:], bv[:])
            bprojsb = pp.tile([128, NC], F32, tag="bprojsb")
            nc.sync.dma_start(bprojsb[:], bproj.rearrange("(c p) -> p c", p=128))
            bfc2sb = pp.tile([128, NC], F32, tag="bfc2sb")
            nc.sync.dma_start(bfc2sb[:], bfc2.rearrange("(c p) -> p c", p=128))

            # V-bias broadcast tile [128 tokens, 768 feats] via K=1 matmul
            vbias = pp.tile([128, C], BF16, tag="vbias")
            vbps = ps.tile([128, C], F32, tag="mm", name="vbps")
            nc.tensor.matmul(vbps[:, 0:512], ones1[0:1, 0:128],
                             bvsb[0:1, 0:512], start=True, stop=True)
            nc.tensor.matmul(vbps[:, 512:C], ones1[0:1, 0:128],
                             bvsb[0:1, 512:C], start=True, stop=True)
            nc.scalar.activation(vbias[:], vbps[:], AF.Copy)

            import contextlib
            loop_cm = (tc.For_i(0, loop_n, 1) if loop_n
                       else contextlib.nullcontext())
            with loop_cm:
              for _rep in range(nrep):
                # ---------------- big persistent activations ----------------
                # x f32 loads half-T first so LN1 stats start at ~4.3us; the
                # second half + wv stream in behind it
                xts = bigp.tile([128, NC, T], F32R, tag="big")     # x^T f32 bits
                xTv = xT.rearrange("(c p) t -> p c t", p=128)
                for n0 in (0, 512):
                    for c in range(NC):
                        nc.sync.dma_start(xts[:, c, n0:n0 + 512],
                                          xTv[:, c, n0:n0 + 512])
                wvsb = bigp.tile([128, NC, C], BF16, tag="big", name="wvsb")
                nc.sync.dma_start(wvsb[:], wv.rearrange("(c p) v -> p c v", p=128))
                x1 = pp.tile([128, NC, T], F32R, tag="x1")         # residual after attn
                lnout = pp.tile([128, NC, T], BF16, tag="lnout")   # LN1 output
                lnf8 = pp.tile([128, NC, T], FP8, tag="lnf8")      # LN2 out fp8 x16
                qT = pp.tile([128, NC, T], BF16, tag="qT")
                kT = pp.tile([128, NC, T], BF16, tag="kT")
                yT = pp.tile([128, NC, T], BF16, tag="yT")
                vsb = pp.tile([128, NT, H, HD + 1], BF16, tag="vsb")
                nc.vector.memset(vsb[:, :, :, HD:HD + 1], 1.0)
                mu_sb = pp.tile([128, T], F32, tag="mu")
                rstd_sb = pp.tile([128, T], F32, tag="rstd")

                # ---------------- helpers ----------------
                def layernorm(src, dst, cen_eng=None, rstd_bias=None):
                    """dst = (src - mean_f) * rsqrt(var_f + eps) [* exp(
                    rstd_bias) when given -> fp8 pre-scale folded into rstd];
                    feature (=partition+chunk) reductions via matmuls with an
                    all-ones/C stationary -> partition-broadcast mean directly.
                    src is an f32r tile (f32 bits): stat matmuls read it
                    directly; element-wise engines read a f32-bitcast view;
                    squares computed on GPSIMD into f32r tiles."""
                    MU = ps.tile([128, T], F32, tag="mm", name="MUp")
                    SSQ = ps.tile([128, T], F32, tag="acc", name="SSQp")
                    for c in range(NC):
                        x2r = frp.tile([128, T], F32R, tag="fr", name=f"x2r{c}")
                        for n0 in (0, 512):
                            hs = slice(n0, n0 + 512)
                            sv = src[:, c, hs].bitcast(F32)
                            nc.gpsimd.tensor_tensor(x2r[:, hs], sv, sv,
                                                    op=OP.mult)
                            nc.tensor.matmul(MU[:, hs], jr[:], src[:, c, hs],
                                             start=(c == 0), stop=(c == NC - 1))
                            nc.tensor.matmul(SSQ[:, hs], jr[:], x2r[:, hs],
                                             start=(c == 0), stop=(c == NC - 1))
                    for n0 in (0, 512):
                        hs = slice(n0, n0 + 512)
                        nc.vector.tensor_copy(mu_sb[:, hs], MU[:, hs])
                        musq = t32p.tile([128, 512], F32, tag="th", name=f"musq{n0}")
                        nc.vector.tensor_tensor(musq[:], mu_sb[:, hs], mu_sb[:, hs],
                                                op=OP.mult)
                        var = t32p.tile([128, 512], F32, tag="th", name=f"var{n0}")
                        nc.vector.tensor_tensor(var[:], SSQ[:, hs], musq[:],
                                                op=OP.subtract)
                        lnv = t32p.tile([128, 512], F32, tag="th", name=f"lnv{n0}")
                        nc.scalar.activation(lnv[:], var[:], AF.Ln, bias=eps_sb[:])
                        if rstd_bias is None:
                            nc.scalar.activation(rstd_sb[:, hs], lnv[:], AF.Exp,
                                                 scale=-0.5)
                        else:
                            nc.scalar.activation(rstd_sb[:, hs], lnv[:], AF.Exp,
                                                 scale=-0.5, bias=rstd_bias)
                        for c in range(NC):
                            cen = t32p.tile([128, 512], F32, tag="th",
                                            name=f"cen{n0}_{c}")
                            (cen_eng or nc.gpsimd).tensor_tensor(
                                cen[:], src[:, c, hs].bitcast(F32),
                                mu_sb[:, hs], op=OP.subtract)
                            nc.vector.tensor_tensor(dst[:, c, hs], cen[:],
                                                    rstd_sb[:, hs], op=OP.mult)

                _mark(nc, 'ln1')
                layernorm(xts, lnout)

                # ---------------- phase 2+3: qkv interleaved with attention ----
                _mark(nc, 'V')
                for ti in range(NT):
                    pm = ps.tile([128, T], F32, tag="mm" if ti % 2 == 0 else "acc", name=f"vp{ti}")
                    for k in range(NC):
                        lh = lnout[:, k, ti * 128:(ti + 1) * 128]
                        nc.tensor.matmul(pm[:, 0:512], lh, wvsb[:, k, 0:512],
                                         start=(k == 0), stop=(k == NC - 1))
                        nc.tensor.matmul(pm[:, 512:768], lh, wvsb[:, k, 512:768],
                                         start=(k == 0), stop=(k == NC - 1))
                    nc.vector.tensor_tensor(
                        vsb[:, ti, :, 0:HD],
                        pm[:, 0:768].rearrange("p (h d) -> p h d", h=H),
                        vbias[:].rearrange("p (h d) -> p h d", h=H),
                        op=OP.add)

                def qk_chunk(mi):
                    # Q^T / K^T feature chunk (feature-major): lhsT = W tile
                    pm = ps.tile([128, T], F32, tag="mm", name=f"qkp{mi}")
                    wt = wp.tile([128, NC, 128], BF16, tag="w", name=f"wqk{mi}")
                    nc.sync.dma_start(wt[:], wqk[mi].rearrange("p (c m) -> p c m", c=NC))
                    for k in range(NC):
                        for n0 in (0, 512):
                            nc.tensor.matmul(pm[:, n0:n0 + 512], wt[:, k, :],
                                             lnout[:, k, n0:n0 + 512],
                                             start=(k == 0), stop=(k == NC - 1))
                    dstt = qT if mi < NC else kT
                    nc.vector.tensor_scalar_add(
                        dstt[:, mi % NC, :], pm[:], bqksb[:, mi:mi + 1])

                def attn_pair(p, yPa, yPb):
                    # two heads (2p: partitions 0:64, 2p+1: 64:128) interleaved
                    # per k-tile: their score matmuls land in different PE row
                    # groups and run concurrently; exp of one head overlaps the
                    # other head's matmuls. For the short k-tiles (qlen<=512)
                    # both heads' scores share one PSUM tile and one exp call.
                    # Causal mask: 0/1 multiply on the post-exp diagonal block
                    # (GPSIMD), so score matmuls need no -1e9 accumulation.
                    ch = p
                    for kt in range(NT):
                        qs = 128 * kt
                        qlen = T - qs
                        pts = []
                        if qlen <= 512:
                            sT2 = ps.tile([128, 2, 512], F32, tag="mm",
                                          name=f"sT2{p}_{kt}")
                            pt2 = ptp.tile([128, 2, 512], BF16, tag="pt",
                                           name=f"pt2{p}_{kt}")
                            for i, p0 in enumerate((0, 64)):
                                lh = kT[p0:p0 + 64, ch, qs:qs + 128]
                                nc.tensor.matmul(sT2[:, i, 0:qlen], lh,
                                                 qT[p0:p0 + 64, ch, qs:T],
                                                 start=True, stop=True)
                            nc.scalar.activation(pt2[:, :, 0:qlen],
                                                 sT2[:, :, 0:qlen], AF.Exp,
                                                 scale=float(1.0 / np.sqrt(HD)))
                            for i in range(2):
                                nc.gpsimd.tensor_tensor(
                                    pt2[:, i, 0:128], pt2[:, i, 0:128],
                                    trimask[:], op=OP.mult)
                                pts.append(pt2[:, i, :])
                        else:
                          sts = []
                          for i, p0 in enumerate((0, 64)):
                            sT = ps.tile([128, T], F32, tag="mm",
                                         name=f"sT{p}_{i}_{kt}")
                            lh = kT[p0:p0 + 64, ch, qs:qs + 128]
                            for a in range(0, qlen, 512):
                                bnd = min(a + 512, qlen)
                                nc.tensor.matmul(sT[:, a:bnd], lh,
                                                 qT[p0:p0 + 64, ch,
                                                    qs + a:qs + bnd],
                                                 start=True, stop=True)
                            sts.append(sT)
                          for i in range(2):
                            pt = ptp.tile([128, T], BF16, tag="pt",
                                          name=f"pt{p}_{i}_{kt}")
                            nc.scalar.activation(pt[:, 0:qlen], sts[i][:, 0:qlen],
                                                 AF.Exp,
                                                 scale=float(1.0 / np.sqrt(HD)))
                            nc.gpsimd.tensor_tensor(
                                pt[:, 0:128], pt[:, 0:128],
                                trimask[:], op=OP.mult)
                            pts.append(pt)
                        for i, yP in enumerate((yPa, yPb)):
                            h = 2 * p + i
                            vl = vsb[:, kt, h, :]
                            pt = pts[i]
                            if qs < 512:
                                nc.tensor.matmul(yP[0:HD + 1, qs:512], vl,
                                                 pt[:, 0:512 - qs],
                                                 start=(kt == 0), stop=(kt == 3))
                                nc.tensor.matmul(yP[0:HD + 1, 512:T], vl,
                                                 pt[:, 512 - qs:T - qs],
                                                 start=(kt == 0),
                                                 stop=(kt == NT - 1))
                            else:
                                nc.tensor.matmul(yP[0:HD + 1, qs:T], vl,
                                                 pt[:, 0:qlen],
                                                 start=False, stop=(kt == NT - 1))

                def attn_norm(h, yP):
                    # rec = 1/denominator via DVE reciprocal (PSUM read);
                    # broadcast over 64 partitions via a K=1 fp32r matmul;
                    # y = y' * rec
                    p0 = 64 * (h % 2)
                    ch = h // 2
                    rec = frp.tile([128, T], F32R, tag="fr", name=f"rec{h}")
                    with nc.allow_low_precision(reason="softmax denom bcast"):
                        nc.vector.reciprocal(rec[HD:HD + 1, :], yP[HD:HD + 1, :])
                    R64 = ps.tile([128, T], F32, tag="mm", name=f"r64_{h}")
                    for n0 in (0, 512):
                        nc.tensor.matmul(R64[0:64, n0:n0 + 512],
                                         o64r[HD:HD + 1, 0:64],
                                         rec[HD:HD + 1, n0:n0 + 512],
                                         start=True, stop=True)
                    r64sb = t32p.tile([128, T], F32, tag="t32", name=f"r64sb{h}")
                    nc.vector.tensor_copy(r64sb[0:64, :], R64[0:64, :])
                    if p0 == 0:
                        nc.vector.tensor_tensor(yT[0:64, ch, :], yP[0:64, :],
                                                r64sb[0:64, :], op=OP.mult)
                    else:
                        yo = ptp.tile([128, T], BF16, tag="pt", name=f"yo{h}")
                        nc.vector.tensor_tensor(yo[0:64, :], yP[0:64, :],
                                                r64sb[0:64, :], op=OP.mult)
                        nc.sync.dma_start(yT[64:128, ch, :], yo[0:64, :])

                # interleave: QK chunk pair p, then attention heads 2p / 2p+1 —
                # the next pair's QK matmuls keep PE busy while ACT runs exp
                pending = None
                for p in range(NC):
                    _mark(nc, f'pair{p}')
                    qk_chunk(p)       # Q chunk p
                    qk_chunk(NC + p)  # K chunk p
                    # normalization of the previous pair lands here so its
                    # reciprocal chain overlaps this pair's QK matmuls on PE
                    if pending is not None:
                        attn_norm(pending[0], pending[1])
                        attn_norm(pending[0] + 1, pending[2])
                    yPa = ps.tile([128, T], F32, tag="acc", name=f"yp{2 * p}")
                    yPb = ps.tile([128, T], F32, tag="acc", name=f"yp{2 * p + 1}")
                    attn_pair(p, yPa, yPb)
                    pending = (2 * p, yPa, yPb)
                if pending is not None:
                    attn_norm(pending[0], pending[1])
                    attn_norm(pending[0] + 1, pending[2])

                _mark(nc, 'proj')
                for m in range(NC):
                    pm = ps.tile([128, T], F32, tag="mm" if m % 2 == 0 else "acc", name=f"pj{m}")
                    wt = wp.tile([128, NC, 128], BF16, tag="w", name=f"wpj{m}")
                    nc.sync.dma_start(wt[:], wproj[m].rearrange("p (c m) -> p c m", c=NC))
                    for k in range(NC):
                        for n0 in (0, 512):
                            nc.tensor.matmul(pm[:, n0:n0 + 512], wt[:, k, :],
                                             yT[:, k, n0:n0 + 512],
                                             start=(k == 0), stop=(k == NC - 1))
                    t = tep.tile([128, T], F32, tag="te", name=f"pjt{m}")
                    nc.scalar.activation(t[:], pm[:], AF.Identity,
                                         bias=bprojsb[:, m:m + 1])
                    nc.gpsimd.tensor_tensor(x1[:, m, :], t[:],
                                            xts[:, m, :].bitcast(F32),
                                            op=OP.add)

                _mark(nc, 'ln2')
                layernorm(x1, lnf8, cen_eng=nc.vector, rstd_bias=ln16_sb)

                # ---------------- phase 6/7: MLP (fp8 DoubleRow) ----------------
                # fc: out = gelu(psum/(16*s_fc) + b); fc2: out = psum/s_fc2 + b
                for half in range(2):
                    _mark(nc, f'mlp{half}')
                    h2 = bigp.tile([128, NF // 2, T], FP8, tag="big",
                                   name=f"h2_{half}")
                    for m in range(NF // 2):
                        mg = half * (NF // 2) + m
                        pm = ps.tile([128, T], F32, tag="mm" if mg % 2 == 0 else "acc", name=f"fc{mg}")
                        wt = wp.tile([128, NC, 128], FP8, tag="w", name=f"wfc{mg}")
                        nc.sync.dma_start(
                            wt[:], wfc[mg].rearrange("p (c m) -> p c m", c=NC))
                        for k in range(NC // 2):
                            for n0 in (0, 512):
                                nc.tensor.matmul(
                                    pm[:, n0:n0 + 512],
                                    wt[:, 2 * k:2 * k + 2, :],
                                    lnf8[:, 2 * k:2 * k + 2, n0:n0 + 512],
                                    start=(k == 0), stop=(k == NC // 2 - 1),
                                    perf_mode=MM8)
                        nc.scalar.activation(h2[:, m, :], pm[:], AF.Gelu,
                                             bias=bfcsb[:, mg:mg + 1],
                                             scale=float(1.0 / (ACT_SCALE * s_fc)))
                    for m in range(NC):
                        pm = ps.tile([128, T], F32, tag="mm" if m % 2 == 0 else "acc", name=f"fc2_{half}_{m}")
                        wt = wp.tile([128, NF // 2, 128], FP8, tag="w",
                                     name=f"wfc2_{half}_{m}")
                        nc.sync.dma_start(
                            wt[:],
                            wfc2[m, :, half * 1536:(half + 1) * 1536]
                            .rearrange("p (c m) -> p c m", c=NF // 2))
                        last = NF // 4 - 1
                        for k in range(NF // 4):
                            for n0 in (0, 512):
                                nc.tensor.matmul(
                                    pm[:, n0:n0 + 512],
                                    wt[:, 2 * k:2 * k + 2, :],
                                    h2[:, 2 * k:2 * k + 2, n0:n0 + 512],
                                    start=(k == 0), stop=(k == last),
                                    perf_mode=MM8)
                        t = tep.tile([128, T], F32, tag="te",
                                      name=f"f2t{half}_{m}")
                        if half == 0:
                            nc.scalar.activation(t[:], pm[:], AF.Identity,
                                                 bias=bfc2sb[:, m:m + 1],
                                                 scale=float(1.0 / s_fc2))
                        else:
                            nc.scalar.activation(t[:], pm[:], AF.Identity,
                                                 scale=float(1.0 / s_fc2))
                        nc.gpsimd.tensor_tensor(x1[:, m, :], t[:],
                                                x1[:, m, :].bitcast(F32),
                                                op=OP.add)
                        if half == 1:
                            nc.sync.dma_start(outT[m * 128:(m + 1) * 128, :],
                                              x1[:, m, :].bitcast(F32))


    _split_excess_waits(nc)
    return nc


_STATE = {}


def kernel(**inputs):
    x = np.asarray(inputs["x"], np.float32)
    ln1_g = np.asarray(inputs["ln1_g"], np.float32)
    ln1_b = np.asarray(inputs["ln1_b"], np.float32)
    ln2_g = np.asarray(inputs["ln2_g"], np.float32)
    ln2_b = np.asarray(inputs["ln2_b"], np.float32)
    W_attn = np.asarray(inputs["W_attn"], np.float32)
    b_attn = np.asarray(inputs["b_attn"], np.float32)
    W_proj = np.asarray(inputs["W_proj"], np.float32)
    b_proj = np.asarray(inputs["b_proj"], np.float32)
    W_fc = np.asarray(inputs["W_fc"], np.float32)
    b_fc = np.asarray(inputs["b_fc"], np.float32)
    W_fc2 = np.asarray(inputs["W_fc2"], np.float32)
    b_fc2 = np.asarray(inputs["b_fc2"], np.float32)

    # fold LN affine into the following matmul (exact): (n*g + b) @ W
    Wa = W_attn * ln1_g[:, None]
    ba = b_attn + ln1_b @ W_attn
    Wf = W_fc * ln2_g[:, None]
    bf = b_fc + ln2_b @ W_fc

    def blk(w, dt=BF):
        # [K, M] -> [M/128 blocks][128 kp][K/128 * 128 mp] with feature
        # f = 128*kc + kp on the partition axis
        K, M = w.shape
        return np.ascontiguousarray(
            w.astype(dt).reshape(K // 128, 128, M // 128, 128)
            .transpose(2, 1, 0, 3).reshape(M // 128, 128, K))

    def pscale(w):
        return float(2.0 ** np.floor(np.log2(224.0 / np.abs(w).max())))

    s_fc = pscale(Wf)
    s_fc2 = pscale(W_fc2)

    def q8(w, s):
        return np.clip(w * s, -240.0, 240.0)

    shared = {
        "wqk": blk(Wa[:, :2 * C]),
        "wv": np.ascontiguousarray(Wa[:, 2 * C:].astype(BF)),
        "bqk": np.ascontiguousarray(ba[:2 * C]),
        "bv": np.ascontiguousarray(ba[None, 2 * C:].astype(BF)),
        "wproj": blk(W_proj),
        "bproj": np.ascontiguousarray(b_proj),
        "wfc": blk(q8(Wf, s_fc), F8),
        "bfc": np.ascontiguousarray(bf),
        "wfc2": blk(q8(W_fc2, s_fc2), F8),
        "bfc2": np.ascontiguousarray(b_fc2),
    }
    in_maps = []
    for b in range(B):
        m = dict(shared)
        m["xT"] = np.ascontiguousarray(x[b].T)
        in_maps.append(m)

    key = ("nc", s_fc, s_fc2)
    if key not in _STATE:
        _STATE[key] = _build(s_fc=s_fc, s_fc2=s_fc2)
    global _last_in_maps
    _last_in_maps = in_maps
    res = run_bass_kernel_spmd(_STATE[key], in_maps, core_ids=list(range(B)))
    out = np.stack([r["outT"].T for r in res.results])
    return np.ascontiguousarray(out, dtype=np.float32)


# revision 44
# speedup vs baseline: 1.3310x; 1.1560x over previous
"""Trainium2 Bass kernel for a GPT-2 transformer block (nn_Block_29343216566701).

Sharding: data-parallel over batch B=8 -> 8 NeuronCores, one batch element per
core, no collectives. Each core runs the full block on [1024 tokens, 768 feats].

On-chip layout is feature-major (x^T: [768, 1024] as [128, 6, 1024] SBUF tiles)
so every matmul contracts over the partition dim without transposes:
  - qkv:   out = W^T @ x^T  (lhsT = W as stored, rhs = x^T)       [feature-major]
  - V:     out = x^T^T @ Wv (lhsT = x^T tile, rhs = Wv)           [token-major]
  - scores:s^T[k,q] (lhsT = k^T head slice, rhs = q^T head)       [k x q]
  - attn:  y' = V'(token-major, +ones col)^T.T @ exp(s^T)         [d+1 x q]
  - denominators come from the ones column; 1/x via DVE reciprocal.
Precision: bf16 matmul inputs with f32 PSUM accumulation; f32 residual spine;
fp32r (1 cycle/row, ~12-bit mantissa) for LN stat matmuls; LN gamma/beta folded
into the following weight matrix host-side (exact for this block).
Engine balance: LN squares + causal mask (0/1 multiply post-exp) on GPSIMD;
V bias via precomputed SBUF tile + DVE add; LN stat matmuls read activation
tiles directly (bf16 or f32 bitcast to f32r) with no staging copies.
A bf16 copy of x is DMA'd first so LN1 stats start ~2x sooner; the f32 x for
the residual spine streams in behind it.
"""
import numpy as np
import ml_dtypes

import concourse.bass as bass
import concourse.tile as tile
import concourse.mybir as mybir
from concourse.bass_utils import run_bass_kernel_spmd
from concourse.vector_clock import ScopedClock

F32 = mybir.dt.float32
F32R = mybir.dt.float32r
BF16 = mybir.dt.bfloat16
FP8 = mybir.dt.float8e4
AF = mybir.ActivationFunctionType
OP = mybir.AluOpType
BF = ml_dtypes.bfloat16
F8 = ml_dtypes.float8_e4m3
MM8 = mybir.MatmulPerfMode.DoubleRow
ACT_SCALE = 16.0          # fp8 activation pre-scale (lnout2, attention v)

B, T, C = 8, 1024, 768
H, HD = 12, 64
NC = C // 128          # 6 feature chunks
NT = T // 128          # 8 token tiles
FF = 4 * C             # 3072
NF = FF // 128         # 24
LN_EPS = 1e-5


# ---------------------------------------------------------------------------
# walrus codegen accepts only one fused semaphore wait per instruction; hoist
# excess waits onto preceding nofuse NOPs on the same engine.
def _split_excess_waits(nc, cap=1):
    for fn in nc.m.functions:
        for bb in fn.blocks:
            new = []
            changed = False
            for ins in bb.instructions:
                si = getattr(ins, "sync_info", None)
                waits = list(si.on_wait) if (si is not None and si.on_wait) else []
                if len(waits) > cap:
                    changed = True
                    for i, w in enumerate(waits[:-cap]):
                        new.append(mybir.InstNoOp(
                            name=f"{ins.name}-w{i}",
                            engine=ins.engine,
                            sync_info=mybir.SyncInfo(on_wait=[w], on_update=[]),
                            bass_nofuse=True,
                        ))
                    ins.sync_info = mybir.SyncInfo(
                        on_wait=waits[-cap:], on_update=list(si.on_update))
                new.append(ins)
            if changed:
                bb.instructions = new


class _SplitDrainTC(tile.TileContext):
    """TileContext whose kernel-tail drain carries its waits on single-wait
    NOPs (the stock version fuses them all onto one drain instruction)."""

    def _drain_and_barrier(self, tick_clock, wait_clock):
        nc = self.nc
        probe = nc.sync.nop(nofuse=True, hint="tail_wait0")
        wait_clock.add_sem_waits(
            probe.ins, ScopedClock({None: tick_clock.global_clock}))
        waits = list(probe.ins.sync_info.on_wait) if probe.ins.sync_info else []
        if len(waits) > 1:
            probe.ins.sync_info = mybir.SyncInfo(on_wait=waits[:1], on_update=[])
            for i, w in enumerate(waits[1:]):
                n = nc.sync.nop(nofuse=True, hint=f"tail_wait{i + 1}")
                n.ins.sync_info = mybir.SyncInfo(on_wait=[w], on_update=[])
        nc.sync.drain()
        nc.all_engine_barrier()
        assert self.sems is not None
        popped = nc._tile_sem_poison_stack.pop()
        assert popped is self._sem_poison
        nc.clear_and_free_semaphores(list(self.sems.allocated().values()))
        nc.all_engine_barrier()


# ---------------------------------------------------------------------------
PHASE_MARKS = []


def _mark(nc, label):
    if not any(l == label for l, _ in PHASE_MARKS):
        PHASE_MARKS.append((label, len(nc.inst_map)))


S_V = 64.0        # net scale on V values (16 from lnout x 4 on wv); the ones
                  # column carries the same 64 so softmax normalization
                  # cancels it exactly — no descale needed anywhere
EXP_BIAS = 2.0    # exp(s/sqrt(hd) + 2): max ~98 < 240, better fp8 range;
                  # cancels in the softmax ratio


def _build(nrep=1, loop_n=0, s_fc=4096.0, s_fc2=8192.0, s_qk=4096.0):
    nc = bass.Bass(trn_type="TRN2", name="gpt2block")
    # q/k are stored scaled by 16*s_qk (lnout x16, wqk x s_qk); the exp scale
    # absorbs the descale: exp(s_raw/(sqrt(hd)*(16 s_qk)^2) + EXP_BIAS)
    exp_scale = float(1.0 / (np.sqrt(HD) * (16.0 * s_qk) ** 2))

    xT = nc.dram_tensor("xT", [C, T], F32R, kind="ExternalInput")
    wqk = nc.dram_tensor("wqk", [2 * NC, 128, C], FP8, kind="ExternalInput")
    wv = nc.dram_tensor("wv", [C, C], FP8, kind="ExternalInput")
    bqk = nc.dram_tensor("bqk", [2 * C], F32, kind="ExternalInput")
    bv = nc.dram_tensor("bv", [1, C], BF16, kind="ExternalInput")
    wproj = nc.dram_tensor("wproj", [NC, 128, C], BF16, kind="ExternalInput")
    bproj = nc.dram_tensor("bproj", [C], F32, kind="ExternalInput")
    wfc = nc.dram_tensor("wfc", [NF, 128, C], FP8, kind="ExternalInput")
    bfc = nc.dram_tensor("bfc", [FF], F32, kind="ExternalInput")
    wfc2 = nc.dram_tensor("wfc2", [NC, 128, FF], FP8, kind="ExternalInput")
    bfc2 = nc.dram_tensor("bfc2", [C], F32, kind="ExternalInput")
    outT = nc.dram_tensor("outT", [C, T], F32, kind="ExternalOutput")

    # causal keep-mask for the diagonal 128x128 block of s^T[k, q]:
    # keep q >= k (upper triangle incl. diagonal); applied post-exp on GPSIMD
    # to both heads of a pair at once ([128, 2, 128] view)
    _tri = np.triu(np.ones((128, 128)))
    trimask_d = nc.inline_tensor(
        np.concatenate([_tri, _tri], axis=1).astype(BF), name="trimask")

    with _SplitDrainTC(nc) as tc:
        with tc.tile_pool(name="persist", bufs=1) as pp, \
             tc.tile_pool(name="big", bufs=2) as bigp, \
             tc.tile_pool(name="t32p", bufs=2) as t32p, \
             tc.tile_pool(name="frp", bufs=2) as frp, \
             tc.tile_pool(name="tep", bufs=3) as tep, \
             tc.tile_pool(name="ptp", bufs=4) as ptp, \
             tc.tile_pool(name="wp", bufs=4) as wp, \
             tc.tile_pool(name="ps", bufs=2, space="PSUM") as ps:

            # ---------------- constants / small inputs ----------------
            trimask2 = pp.tile([128, 2, 128], BF16, tag="trimask")
            nc.sync.dma_start(trimask2[:],
                              trimask_d.rearrange("p (j m) -> p j m", j=2))
            ones1 = pp.tile([1, 128], BF16, tag="ones1")
            nc.vector.memset(ones1[:], 1.0)
            j32 = t32p.tile([128, 128], F32, tag="t32", name="j32")
            nc.vector.memset(j32[:], 1.0 / C)
            jr = pp.tile([128, 128], F32R, tag="jr")
            nc.vector.tensor_copy(jr[:], j32[:])
            o32 = t32p.tile([128, 128], F32, tag="t32", name="o32")
            nc.vector.memset(o32[:], 1.0)
            o64r = pp.tile([128, 128], F32R, tag="o64r")
            nc.vector.tensor_copy(o64r[:], o32[:])
            eps_sb = pp.tile([128, 1], F32, tag="eps")
            nc.vector.memset(eps_sb[:], LN_EPS)
            ln16_sb = pp.tile([128, 1], F32, tag="ln16")
            nc.vector.memset(ln16_sb[:], float(np.log(ACT_SCALE)))
            eb_sb = pp.tile([128, 1], F32, tag="eb")
            nc.vector.memset(eb_sb[:], float(EXP_BIAS))

            bqksb = pp.tile([128, 2 * NC], F32, tag="bqksb")
            nc.sync.dma_start(bqksb[:], bqk.rearrange("(c p) -> p c", p=128))
            bfcsb = pp.tile([128, NF], F32, tag="bfcsb")
            nc.sync.dma_start(bfcsb[:], bfc.rearrange("(c p) -> p c", p=128))
            bvsb = t32p.tile([1, C], BF16, tag="t32", name="bvsb")
            nc.sync.dma_start(bvsb[:], bv[:])
            bprojsb = pp.tile([128, NC], F32, tag="bprojsb")
            nc.sync.dma_start(bprojsb[:], bproj.rearrange("(c p) -> p c", p=128))
            bfc2sb = pp.tile([128, NC], F32, tag="bfc2sb")
            nc.sync.dma_start(bfc2sb[:], bfc2.rearrange("(c p) -> p c", p=128))

            # V-bias broadcast tile [128 tokens, 768 feats] via K=1 matmul
            vbias = pp.tile([128, C], BF16, tag="vbias")
            vbps = ps.tile([128, C], F32, tag="mm", name="vbps")
            nc.tensor.matmul(vbps[:, 0:512], ones1[0:1, 0:128],
                             bvsb[0:1, 0:512], start=True, stop=True)
            nc.tensor.matmul(vbps[:, 512:C], ones1[0:1, 0:128],
                             bvsb[0:1, 512:C], start=True, stop=True)
            nc.scalar.activation(vbias[:], vbps[:], AF.Copy)

            import contextlib
            loop_cm = (tc.For_i(0, loop_n, 1) if loop_n
                       else contextlib.nullcontext())
            with loop_cm:
              for _rep in range(nrep):
                # ---------------- big persistent activations ----------------
                # x f32 loads half-T first so LN1 stats start at ~4.3us; the
                # second half + wv stream in behind it
                xts = bigp.tile([128, NC, T], F32R, tag="big")     # x^T f32 bits
                xTv = xT.rearrange("(c p) t -> p c t", p=128)
                for n0 in (0, 512):
                    for c in range(NC):
                        nc.sync.dma_start(xts[:, c, n0:n0 + 512],
                                          xTv[:, c, n0:n0 + 512])
                wvsb = bigp.tile([128, NC, C], FP8, tag="big", name="wvsb")
                nc.sync.dma_start(wvsb[:], wv.rearrange("(c p) v -> p c v", p=128))
                x1 = pp.tile([128, NC, T], F32R, tag="x1")         # residual after attn
                lnf8 = pp.tile([128, NC, T], FP8, tag="lnf8")      # LN out fp8 x16
                qT = pp.tile([128, NC, T], BF16, tag="qT")
                kT = pp.tile([128, NC, T], BF16, tag="kT")
                yT = pp.tile([128, NC, T], BF16, tag="yT")
                # head stride padded to 68 so the DoubleRow weight AP's
                # k-tile step (12*68 = 816 bytes) is 16-aligned
                vsb = pp.tile([128, NT, H, HD + 4], FP8, tag="vsb")
                nc.vector.memset(vsb[:, :, :, HD:HD + 1], S_V)
                mu_sb = pp.tile([128, T], F32, tag="mu")
                rstd_sb = pp.tile([128, T], F32, tag="rstd")

                # ---------------- helpers ----------------
                def layernorm(src, dst, cen_eng=None, rstd_bias=None):
                    """dst = (src - mean_f) * rsqrt(var_f + eps) [* exp(
                    rstd_bias) when given -> fp8 pre-scale folded into rstd];
                    feature (=partition+chunk) reductions via matmuls with an
                    all-ones/C stationary -> partition-broadcast mean directly.
                    src is an f32r tile (f32 bits): stat matmuls read it
                    directly; element-wise engines read a f32-bitcast view;
                    squares computed on GPSIMD into f32r tiles."""
                    MU = ps.tile([128, T], F32, tag="mm", name="MUp")
                    SSQ = ps.tile([128, T], F32, tag="acc", name="SSQp")
                    for c in range(NC):
                        x2r = frp.tile([128, T], F32R, tag="fr", name=f"x2r{c}")
                        for n0 in (0, 512):
                            hs = slice(n0, n0 + 512)
                            sv = src[:, c, hs].bitcast(F32)
                            nc.gpsimd.tensor_tensor(x2r[:, hs], sv, sv,
                                                    op=OP.mult)
                            nc.tensor.matmul(MU[:, hs], jr[:], src[:, c, hs],
                                             start=(c == 0), stop=(c == NC - 1))
                            nc.tensor.matmul(SSQ[:, hs], jr[:], x2r[:, hs],
                                             start=(c == 0), stop=(c == NC - 1))
                    for n0 in (0, 512):
                        hs = slice(n0, n0 + 512)
                        nc.vector.tensor_copy(mu_sb[:, hs], MU[:, hs])
                        musq = t32p.tile([128, 512], F32, tag="th", name=f"musq{n0}")
                        nc.vector.tensor_tensor(musq[:], mu_sb[:, hs], mu_sb[:, hs],
                                                op=OP.mult)
                        var = t32p.tile([128, 512], F32, tag="th", name=f"var{n0}")
                        nc.vector.tensor_tensor(var[:], SSQ[:, hs], musq[:],
                                                op=OP.subtract)
                        lnv = t32p.tile([128, 512], F32, tag="th", name=f"lnv{n0}")
                        nc.scalar.activation(lnv[:], var[:], AF.Ln, bias=eps_sb[:])
                        if rstd_bias is None:
                            nc.scalar.activation(rstd_sb[:, hs], lnv[:], AF.Exp,
                                                 scale=-0.5)
                        else:
                            nc.scalar.activation(rstd_sb[:, hs], lnv[:], AF.Exp,
                                                 scale=-0.5, bias=rstd_bias)
                        for c in range(NC):
                            cen = t32p.tile([128, 512], F32, tag="th",
                                            name=f"cen{n0}_{c}")
                            (cen_eng or nc.gpsimd).tensor_tensor(
                                cen[:], src[:, c, hs].bitcast(F32),
                                mu_sb[:, hs], op=OP.subtract)
                            nc.vector.tensor_tensor(dst[:, c, hs], cen[:],
                                                    rstd_sb[:, hs], op=OP.mult)

                _mark(nc, 'ln1')
                layernorm(xts, lnf8, rstd_bias=ln16_sb)

                # ---------------- phase 2+3: qkv interleaved with attention ----
                _mark(nc, 'V')
                for ti in range(NT):
                    pm = ps.tile([128, T], F32, tag="mm" if ti % 2 == 0 else "acc", name=f"vp{ti}")
                    for k in range(NC // 2):
                        lh = lnf8[:, 2 * k:2 * k + 2, ti * 128:(ti + 1) * 128]
                        nc.tensor.matmul(pm[:, 0:512], lh,
                                         wvsb[:, 2 * k:2 * k + 2, 0:512],
                                         start=(k == 0), stop=(k == NC // 2 - 1),
                                         perf_mode=MM8)
                        nc.tensor.matmul(pm[:, 512:768], lh,
                                         wvsb[:, 2 * k:2 * k + 2, 512:768],
                                         start=(k == 0), stop=(k == NC // 2 - 1),
                                         perf_mode=MM8)
                    nc.vector.tensor_tensor(
                        vsb[:, ti, :, 0:HD],
                        pm[:, 0:768].rearrange("p (h d) -> p h d", h=H),
                        vbias[:].rearrange("p (h d) -> p h d", h=H),
                        op=OP.add)

                def qk_chunk(mi):
                    # Q^T / K^T feature chunk (feature-major): lhsT = W tile
                    pm = ps.tile([128, T], F32, tag="mm", name=f"qkp{mi}")
                    wt = wp.tile([128, NC, 128], FP8, tag="w", name=f"wqk{mi}")
                    nc.sync.dma_start(wt[:], wqk[mi].rearrange("p (c m) -> p c m", c=NC))
                    for k in range(NC // 2):
                        for n0 in (0, 512):
                            nc.tensor.matmul(pm[:, n0:n0 + 512],
                                             wt[:, 2 * k:2 * k + 2, :],
                                             lnf8[:, 2 * k:2 * k + 2, n0:n0 + 512],
                                             start=(k == 0),
                                             stop=(k == NC // 2 - 1),
                                             perf_mode=MM8)
                    dstt = qT if mi < NC else kT
                    nc.vector.tensor_scalar_add(
                        dstt[:, mi % NC, :], pm[:], bqksb[:, mi:mi + 1])

                def attn_pair(p, yPa, yPb):
                    # Heads 2p (PSUM yPa) and 2p+1 (yPb); scores for both heads
                    # of k-tile kt land in one [128, 2, 512] PSUM tile per
                    # q-segment (row groups 0:64 / 64:128 run concurrently).
                    # exp -> PT fp8 [128, head, kt-parity, q-absolute]; the
                    # diagonal 128 cols get a 0/1 triu multiply on GPSIMD.
                    # attnV contracts two k-tiles per matmul via fp8 DoubleRow
                    # (V and the ones/64 column as [128, 2, 65] stationary),
                    # emitted one kt-pair behind the scores so exp overlaps PE.
                    ch = p

                    def scores_kt(PT, j, kt):
                        qs = 128 * kt
                        segs = ([(qs, 512 - qs), (512, 512)] if qs < 512
                                else [(qs, T - qs)])
                        for q0, w in segs:
                            sT2 = ps.tile([128, 2, 512], F32, tag="mm",
                                          name=f"sT{p}_{kt}_{q0}")
                            for i, p0 in enumerate((0, 64)):
                                nc.tensor.matmul(
                                    sT2[:, i, 0:w],
                                    kT[p0:p0 + 64, ch, qs:qs + 128],
                                    qT[p0:p0 + 64, ch, q0:q0 + w],
                                    start=True, stop=True)
                            nc.scalar.activation(PT[:, :, j, q0:q0 + w],
                                                 sT2[:, :, 0:w], AF.Exp,
                                                 scale=exp_scale, bias=eb_sb[:])
                        nc.gpsimd.tensor_tensor(
                            PT[:, :, j, qs:qs + 128],
                            PT[:, :, j, qs:qs + 128], trimask2[:], op=OP.mult)

                    def attn_v(PT, t):
                        # kt pair (2t, 2t+1); bank0 = cols [0:512), bank1 =
                        # [512:1024); start clears the bank on its first
                        # matmul, stop on its last
                        qs = 256 * t
                        for i, yP in enumerate((yPa, yPb)):
                            h = 2 * p + i
                            vl1 = vsb[:, 2 * t, h, 0:HD + 1]
                            vlp = vsb[:, 2 * t:2 * t + 2, h, 0:HD + 1]
                            # D1: [qs, qs+128) — first k-tile only
                            nc.tensor.matmul(yP[0:HD + 1, qs:qs + 128], vl1,
                                             PT[:, i, 0, qs:qs + 128],
                                             start=(t == 0), stop=False,
                                             skip_group_check=True)
                            # D2: [qs+128, qs+256) — both k-tiles (DoubleRow)
                            nc.tensor.matmul(yP[0:HD + 1, qs + 128:qs + 256],
                                             vlp, PT[:, i, :, qs + 128:qs + 256],
                                             start=False, stop=(t in (1, 3)),
                                             perf_mode=MM8,
                                             skip_group_check=True)
                            # rest of bank0 for t=0: [512-256=256:512)
                            if t == 0:
                                nc.tensor.matmul(yP[0:HD + 1, 256:512], vlp,
                                                 PT[:, i, :, 256:512],
                                                 start=False, stop=False,
                                                 perf_mode=MM8,
                                                 skip_group_check=True)
                            # bank1 512-wide contributions for t=0,1
                            if t in (0, 1):
                                nc.tensor.matmul(yP[0:HD + 1, 512:T], vlp,
                                                 PT[:, i, :, 512:T],
                                                 start=(t == 0), stop=False,
                                                 perf_mode=MM8,
                                                 skip_group_check=True)
                            # t=2 remainder of bank1: [768:1024)
                            if t == 2:
                                nc.tensor.matmul(yP[0:HD + 1, 768:T], vlp,
                                                 PT[:, i, :, 768:T],
                                                 start=False, stop=False,
                                                 perf_mode=MM8,
                                                 skip_group_check=True)

                    pend = None
                    for t in range(4):
                        PT = ptp.tile([128, 2, 2, T], FP8, tag="pt",
                                      name=f"PT{p}_{t}")
                        scores_kt(PT, 0, 2 * t)
                        scores_kt(PT, 1, 2 * t + 1)
                        if pend is not None:
                            attn_v(*pend)
                        pend = (PT, t)
                    attn_v(*pend)

                def attn_norm(h, yP):
                    # rec = 1/denominator via DVE reciprocal (PSUM read);
                    # broadcast over 64 partitions via a K=1 fp32r matmul;
                    # y = y' * rec
                    p0 = 64 * (h % 2)
                    ch = h // 2
                    rec = frp.tile([128, T], F32R, tag="fr", name=f"rec{h}")
                    with nc.allow_low_precision(reason="softmax denom bcast"):
                        nc.vector.reciprocal(rec[HD:HD + 1, :], yP[HD:HD + 1, :])
                    R64 = ps.tile([128, T], F32, tag="mm", name=f"r64_{h}")
                    for n0 in (0, 512):
                        nc.tensor.matmul(R64[0:64, n0:n0 + 512],
                                         o64r[HD:HD + 1, 0:64],
                                         rec[HD:HD + 1, n0:n0 + 512],
                                         start=True, stop=True)
                    r64sb = t32p.tile([128, T], F32, tag="t32", name=f"r64sb{h}")
                    nc.vector.tensor_copy(r64sb[0:64, :], R64[0:64, :])
                    if p0 == 0:
                        nc.vector.tensor_tensor(yT[0:64, ch, :], yP[0:64, :],
                                                r64sb[0:64, :], op=OP.mult)
                    else:
                        yo = ptp.tile([128, T], BF16, tag="pt", name=f"yo{h}")
                        nc.vector.tensor_tensor(yo[0:64, :], yP[0:64, :],
                                                r64sb[0:64, :], op=OP.mult)
                        nc.sync.dma_start(yT[64:128, ch, :], yo[0:64, :])

                # interleave: QK chunk pair p, then attention heads 2p / 2p+1 —
                # the next pair's QK matmuls keep PE busy while ACT runs exp
                pending = None
                for p in range(NC):
                    _mark(nc, f'pair{p}')
                    qk_chunk(p)       # Q chunk p
                    qk_chunk(NC + p)  # K chunk p
                    # normalization of the previous pair lands here so its
                    # reciprocal chain overlaps this pair's QK matmuls on PE
                    if pending is not None:
                        attn_norm(pending[0], pending[1])
                        attn_norm(pending[0] + 1, pending[2])
                    yPa = ps.tile([128, T], F32, tag="acc", name=f"yp{2 * p}")
                    yPb = ps.tile([128, T], F32, tag="acc", name=f"yp{2 * p + 1}")
                    attn_pair(p, yPa, yPb)
                    pending = (2 * p, yPa, yPb)
                if pending is not None:
                    attn_norm(pending[0], pending[1])
                    attn_norm(pending[0] + 1, pending[2])

                _mark(nc, 'proj')
                for m in range(NC):
                    pm = ps.tile([128, T], F32, tag="mm" if m % 2 == 0 else "acc", name=f"pj{m}")
                    wt = wp.tile([128, NC, 128], BF16, tag="w", name=f"wpj{m}")
                    nc.sync.dma_start(wt[:], wproj[m].rearrange("p (c m) -> p c m", c=NC))
                    for k in range(NC):
                        for n0 in (0, 512):
                            nc.tensor.matmul(pm[:, n0:n0 + 512], wt[:, k, :],
                                             yT[:, k, n0:n0 + 512],
                                             start=(k == 0), stop=(k == NC - 1))
                    t = tep.tile([128, T], F32, tag="te", name=f"pjt{m}")
                    nc.scalar.activation(t[:], pm[:], AF.Identity,
                                         bias=bprojsb[:, m:m + 1])
                    nc.gpsimd.tensor_tensor(x1[:, m, :], t[:],
                                            xts[:, m, :].bitcast(F32),
                                            op=OP.add)

                _mark(nc, 'ln2')
                layernorm(x1, lnf8, cen_eng=nc.vector, rstd_bias=ln16_sb)

                # ---------------- phase 6/7: MLP (fp8 DoubleRow) ----------------
                # fc: out = gelu(psum/(16*s_fc) + b); fc2: out = psum/s_fc2 + b
                for half in range(2):
                    _mark(nc, f'mlp{half}')
                    h2 = bigp.tile([128, NF // 2, T], FP8, tag="big",
                                   name=f"h2_{half}")
                    for m in range(NF // 2):
                        mg = half * (NF // 2) + m
                        pm = ps.tile([128, T], F32, tag="mm" if mg % 2 == 0 else "acc", name=f"fc{mg}")
                        wt = wp.tile([128, NC, 128], FP8, tag="w", name=f"wfc{mg}")
                        nc.sync.dma_start(
                            wt[:], wfc[mg].rearrange("p (c m) -> p c m", c=NC))
                        for k in range(NC // 2):
                            for n0 in (0, 512):
                                nc.tensor.matmul(
                                    pm[:, n0:n0 + 512],
                                    wt[:, 2 * k:2 * k + 2, :],
                                    lnf8[:, 2 * k:2 * k + 2, n0:n0 + 512],
                                    start=(k == 0), stop=(k == NC // 2 - 1),
                                    perf_mode=MM8)
                        nc.scalar.activation(h2[:, m, :], pm[:], AF.Gelu,
                                             bias=bfcsb[:, mg:mg + 1],
                                             scale=float(1.0 / (ACT_SCALE * s_fc)))
                    for m in range(NC):
                        pm = ps.tile([128, T], F32, tag="mm" if m % 2 == 0 else "acc", name=f"fc2_{half}_{m}")
                        wt = wp.tile([128, NF // 2, 128], FP8, tag="w",
                                     name=f"wfc2_{half}_{m}")
                        nc.sync.dma_start(
                            wt[:],
                            wfc2[m, :, half * 1536:(half + 1) * 1536]
                            .rearrange("p (c m) -> p c m", c=NF // 2))
                        last = NF // 4 - 1
                        for k in range(NF // 4):
                            for n0 in (0, 512):
                                nc.tensor.matmul(
                                    pm[:, n0:n0 + 512],
                                    wt[:, 2 * k:2 * k + 2, :],
                                    h2[:, 2 * k:2 * k + 2, n0:n0 + 512],
                                    start=(k == 0), stop=(k == last),
                                    perf_mode=MM8)
                        t = tep.tile([128, T], F32, tag="te",
                                      name=f"f2t{half}_{m}")
                        if half == 0:
                            nc.scalar.activation(t[:], pm[:], AF.Identity,
                                                 bias=bfc2sb[:, m:m + 1],
                                                 scale=float(1.0 / s_fc2))
                        else:
                            nc.scalar.activation(t[:], pm[:], AF.Identity,
                                                 scale=float(1.0 / s_fc2))
                        nc.gpsimd.tensor_tensor(x1[:, m, :], t[:],
                                                x1[:, m, :].bitcast(F32),
                                                op=OP.add)
                        if half == 1:
                            nc.sync.dma_start(outT[m * 128:(m + 1) * 128, :],
                                              x1[:, m, :].bitcast(F32))


    _split_excess_waits(nc)
    return nc


_STATE = {}


def kernel(**inputs):
    x = np.asarray(inputs["x"], np.float32)
    ln1_g = np.asarray(inputs["ln1_g"], np.float32)
    ln1_b = np.asarray(inputs["ln1_b"], np.float32)
    ln2_g = np.asarray(inputs["ln2_g"], np.float32)
    ln2_b = np.asarray(inputs["ln2_b"], np.float32)
    W_attn = np.asarray(inputs["W_attn"], np.float32)
    b_attn = np.asarray(inputs["b_attn"], np.float32)
    W_proj = np.asarray(inputs["W_proj"], np.float32)
    b_proj = np.asarray(inputs["b_proj"], np.float32)
    W_fc = np.asarray(inputs["W_fc"], np.float32)
    b_fc = np.asarray(inputs["b_fc"], np.float32)
    W_fc2 = np.asarray(inputs["W_fc2"], np.float32)
    b_fc2 = np.asarray(inputs["b_fc2"], np.float32)

    # fold LN affine into the following matmul (exact): (n*g + b) @ W
    Wa = W_attn * ln1_g[:, None]
    ba = b_attn + ln1_b @ W_attn
    Wf = W_fc * ln2_g[:, None]
    bf = b_fc + ln2_b @ W_fc

    def blk(w, dt=BF):
        # [K, M] -> [M/128 blocks][128 kp][K/128 * 128 mp] with feature
        # f = 128*kc + kp on the partition axis
        K, M = w.shape
        return np.ascontiguousarray(
            w.astype(dt).reshape(K // 128, 128, M // 128, 128)
            .transpose(2, 1, 0, 3).reshape(M // 128, 128, K))

    def pscale(w):
        return float(2.0 ** np.floor(np.log2(224.0 / np.abs(w).max())))

    s_fc = pscale(Wf)
    s_fc2 = pscale(W_fc2)
    s_qk = pscale(Wa[:, :2 * C])

    def q8(w, s):
        return np.clip(w * s, -240.0, 240.0)

    shared = {
        "wqk": blk(q8(Wa[:, :2 * C], s_qk), F8),
        "wv": np.ascontiguousarray(q8(Wa[:, 2 * C:], S_V / ACT_SCALE).astype(F8)),
        "bqk": np.ascontiguousarray(ba[:2 * C] * (ACT_SCALE * s_qk)),
        "bv": np.ascontiguousarray(S_V * ba[None, 2 * C:].astype(np.float32)).astype(BF),
        "wproj": blk(W_proj),
        "bproj": np.ascontiguousarray(b_proj),
        "wfc": blk(q8(Wf, s_fc), F8),
        "bfc": np.ascontiguousarray(bf),
        "wfc2": blk(q8(W_fc2, s_fc2), F8),
        "bfc2": np.ascontiguousarray(b_fc2),
    }
    in_maps = []
    for b in range(B):
        m = dict(shared)
        m["xT"] = np.ascontiguousarray(x[b].T)
        in_maps.append(m)

    key = ("nc", s_fc, s_fc2, s_qk)
    if key not in _STATE:
        _STATE[key] = _build(s_fc=s_fc, s_fc2=s_fc2, s_qk=s_qk)
    global _last_in_maps
    _last_in_maps = in_maps
    res = run_bass_kernel_spmd(_STATE[key], in_maps, core_ids=list(range(B)))
    out = np.stack([r["outT"].T for r in res.results])
    return np.ascontiguousarray(out, dtype=np.float32)
